# revision 75
# baseline (speedup 1.0000x reference)
"""AssimilationLoss Trainium2 kernel.

Reference math (x: [B, N, D] f32):
    loss = mean_b || sum_i x[b,i,:] / max(||x[b,i,:]||, eps) ||^2 / N^2

Sharding: data-parallel over B across 8 NeuronCores (one batch element per
core).  Each core streams its [N, D] shard once from HBM (16 MiB), computes
partial_b = || sum_i x_i/||x_i|| ||^2 locally, and the host averages the 8
scalars.

Timing model (trace-derived):
  - graded window = first MEMSET (framework const setup) .. last trace event
  - the walrus codegen epilogue zeroes all ~253 semaphores behind a barrier
    (~7 us, fixed) -> total = (last engine finish) + ~7.5 us
  - wire: SWDGE saturates 330-430 GB/s (run-to-run DVFS variance); SWDGE's
    first packet lags its first trigger by ~2.4 us, so one 1-tile HWDGE DMA
    on the sync ring covers the warmup gap (more would be starved once the
    SW queue ramps; Q1 gets almost no bandwidth next to a busy Q0).

Per-core pipeline over [128, 512] row-tiles (raw Bacc, manual semaphores):
  DMA : 1-tile HWDGE (f32r) on sync first, then SWDGE chunks from gpsimd
        with f32->bf16 cast on the wire; 4-tile chunks mid-stream (engines
        cannot start a tile until its whole chunk's semaphore fires, so
        8-tile chunks quantize the pipeline into ~5 us bursts), 2,2,1,1,1,1
        at the tail for fine wait granularity.
  ACT : activation(Square, accum_out) -> ss[p] for 7/16 of the tiles
        (a square costs ~985-1180 ns incl. the mandatory accumulator-read;
        ACT also carries the sqrt batches and the final square)
  DVE : affine_mul_reduce -> ss[p] for 9/16 (~770-910 ns/tile)
  ACT : sqrt (batched per DMA chunk)  -> norm[p]
  DVE : reciprocal                    -> inv[p] = 1/||x_p|| (bf16 / f32r)
  PE  : matmul(lhsT=inv, rhs=x_tile)  -> s[1, D] += sum_p x[p,:]/||x_p||
Epilogue: ACT square+acc of s -> scalar, DMA out from the warm sync ring.
Engine-op durations and the wire rate vary ~20-30% run to run (DVFS /
HBM-contention lottery: wire 313-430 GB/s, amr 656-824 ns); measured best
windows ~57.3 us at 410 GB/s.

Synchronization rules (hard-won):
  - DVE affine_mul_reduce accumulator results must be signalled by a LATER
    DVE instruction (engine_nop), never by then_inc on the amr itself, and
    never consumed by the next DVE instruction (no same-engine interlock).
  - sqrt/recip batches are emitted per chunk AFTER that chunk's squares on
    the same engine; a batch emitted after the NEXT chunk's squares sits
    behind data-gated instructions and adds a full chunk (~5 us) of inv
    latency (measured: PE starved ~4.8 us at every group boundary).
"""

import numpy as np

import concourse.bacc as bacc
import concourse.mybir as mybir
from concourse.bass_utils import run_bass_kernel_spmd


def _ensure_ntff_hook():
    """Provide antenv.axon_hooks (NTFF profiling glue) if the image lacks it."""
    try:
        from antenv.axon_hooks import get_axon_ntff_profile_hook  # noqa: F401

        return
    except ImportError:
        pass
    import contextlib
    import ctypes
    import sys
    import types

    so_path = "/opt/axon/libaxon_pjrt.so"
    mod = types.ModuleType("antenv.axon_hooks")
    _state = {"hook": None}
    mod.set_axon_ntff_profile_hook = lambda h: _state.__setitem__("hook", h)
    mod.get_axon_ntff_profile_hook = lambda: _state["hook"]
    try:
        lib = ctypes.CDLL(so_path)
        if hasattr(lib, "axon_start_nrt_profile"):
            lib.axon_start_nrt_profile.argtypes = [
                ctypes.POINTER(ctypes.c_int64),
                ctypes.c_size_t,
            ]
            lib.axon_start_nrt_profile.restype = ctypes.c_int64
            lib.axon_stop_nrt_profile.argtypes = [ctypes.c_char_p]
            lib.axon_stop_nrt_profile.restype = ctypes.c_int64

            @contextlib.contextmanager
            def _hook(output_dir, device_ids):
                import jax

                jax.devices()
                if device_ids:
                    ids = (ctypes.c_int64 * len(device_ids))(*device_ids)
                    rc = lib.axon_start_nrt_profile(ids, len(device_ids))
                else:
                    rc = lib.axon_start_nrt_profile(None, 0)
                if rc != 0:
                    raise RuntimeError(f"axon_start_nrt_profile rc={rc}")
                try:
                    yield
                finally:
                    n = lib.axon_stop_nrt_profile(str(output_dir).encode())
                    if n <= 0:
                        print(f"ntff profile: rc={n} (no files?)", file=sys.stderr)

            _state["hook"] = _hook
    except OSError:
        pass
    import antenv

    sys.modules["antenv.axon_hooks"] = mod
    antenv.axon_hooks = mod


_ensure_ntff_hook()

B, N, D = 8, 8192, 512
P = 128  # SBUF partitions

F32 = mybir.dt.float32
F32R = mybir.dt.float32r
BF16 = mybir.dt.bfloat16

# DMA plan: (n_tiles, kind).  "hs" = HWDGE from sync (f32r storage; the sync
# ring's packets start ~2.2 us before SWDGE's first packet, so a 1-tile hs
# chunk covers the SWDGE warmup gap and gives compute an early start).
# "sw" = SWDGE from gpsimd, f32 -> bf16 cast on the wire.
DMA_PLAN = (
    [(1, "hs")] * 2
    + [(2, "sw")]
    + [(4, "sw")] * 13
    + [(2, "sw")] * 2
    + [(1, "sw")] * 4
)
# Two 1-tile hs chunks: both complete (~2.8/3.5 us) before the SW queue's
# first packet (~4.4 us), so they never get starved by Q0 (3-4 hs tiles
# measured starved to ~19 us once Q0 ramps).  A single 2-tile hs chunk
# measured inconclusive (slow-clock draws); kept at the measured best.
# Tail: 2,2,1,1,1,1.  All-singles measured WORSE (the per-group fixed costs
# -- sqrt batch ~290 ns + recip ~170 ns + nop -- outweigh the finer wait
# granularity when applied to 8 trailing tiles).
# All-small chunks: engines cannot start a tile until its WHOLE chunk's DMA
# semaphore fires, so an 8-tile chunk quantizes the pipeline to ~5-6 us
# bursts (measured ~7 us of matmul lag past the last byte).  4-tile chunks
# halve that; the gpsimd trigger rate (~0.7 us each, 21 triggers) still
# keeps descriptor supply ~3x ahead of the wire.


ACT_SLOTS = (1, 3, 5, 7, 9, 11, 14)  # 7 of every 16 tiles on ACT

# Tiles offloaded to GpSimd: EMPTY.  scalar_tensor_tensor+accum_out looked
# like a third square engine (gpsimd idles from ~20 us), but walrus codegen
# rejects TensorScalar on Pool (NCC_IXCG966: not in the TRN2 Pool ISA); the
# gpsimd only runs its ucode library ops.  Kept as documentation.
GP_TILES = frozenset()


def _engine(t, nt):
    """'act' | 'dve' | 'gp' for tile t's square+rowsum.  An ACT square
    costs ~985-1180 ns (square + mandatory 334 ns accumulator-read) vs
    ~770-910 ns for a DVE affine_mul_reduce, and ACT also carries the sqrt
    batches and the final square, so ACT takes 7/16 of the tiles.  The last
    tile is special-cased: its square is half-split across ACT+DVE."""
    if t == nt - 1:
        return "dve"  # handled by the half-split path
    if t == nt - 2:
        return "dve"  # tail rebalance: ACT is the endgame bottleneck
        #               (tail squares + sqrt batches + final square)
    if t in GP_TILES:
        return "gp"
    return "act" if t % 16 in ACT_SLOTS else "dve"


def _build_nc():
    nc = bacc.Bacc("TRN2", target_bir_lowering=False, debug=False)
    x_ext = nc.dram_tensor("x", [N, D], F32R, kind="ExternalInput")
    out_ext = nc.dram_tensor("out", [1, 1], F32, kind="ExternalOutput")
    _body_raw(nc, x_ext.ap(), out_ext.ap())
    nc.compile()
    return nc


def _body_raw(nc, x, out):
    assert sum(m for m, _ in DMA_PLAN) * P == N

    # per-DMA sbuf storage + tile map
    dmas = []  # (kind, ap, row0, m)
    tiles = []  # (dma_idx, i_in_dma, ap, kind)
    r0 = 0
    for di, (m, kind) in enumerate(DMA_PLAN):
        dt = BF16 if kind == "sw" else F32R
        ap = nc.alloc_sbuf_tensor(f"xt{di}", [P, m, D], dt).ap()
        dmas.append((kind, ap, r0, m))
        for i in range(m):
            tiles.append((di, i, ap, kind))
        r0 += m * P
    assert r0 == N
    NT = len(tiles)

    # sqrt/recip groups: one group per DMA chunk, so a group's sqrt/recip is
    # emitted right after that chunk's squares and never sits behind a
    # data-gated instruction of a later chunk.  The tail 1-tile chunks give
    # per-tile groups, so each tail tile's inv (and matmul) fires as soon as
    # its bytes land.
    groups = []  # (tile0, gsize, kind)
    t = 0
    for m, kind in DMA_PLAN:
        groups.append((t, m, kind))
        t += m
    assert t == NT
    # merge the first pair of trailing 1-tile groups (tiles NT-4, NT-3):
    # one fewer sqrt+recip batch (~0.5 us off the ACT/DVE endgame) for
    # +0.6 us of inv latency on tile NT-4, which PE's backlog absorbs.
    # The last two tiles keep 1-tile groups for minimal tail latency.
    if (
        len(groups) >= 4
        and groups[-4][1] == 1
        and groups[-3][1] == 1
        and groups[-4][2] == groups[-3][2]
    ):
        g4, g3 = groups[-4], groups[-3]
        groups[-4:-2] = [(g4[0], 2, g4[2])]
    # (merging the trailing 2-tile chunk groups as well measured neutral to
    # slightly worse -- the extra inv latency there is not fully absorbed)

    warm = nc.alloc_sbuf_tensor("warm", [1, 1, D], BF16).ap()
    ss = nc.alloc_sbuf_tensor("ss", [P, NT], F32).ap()
    nrm = nc.alloc_sbuf_tensor("nrm", [P, NT], F32).ap()
    inv_r = nc.alloc_sbuf_tensor("inv_r", [P, NT], F32R).ap()
    inv_b = nc.alloc_sbuf_tensor("inv_b", [P, NT], BF16).ap()
    ss_b = nc.alloc_sbuf_tensor("ss_b", [P, 1], F32).ap()
    sq_a = nc.alloc_sbuf_tensor("sq_a", [P, D], F32).ap()
    sq_v = nc.alloc_sbuf_tensor("sq_v", [P, D], F32).ap()
    sq_g = nc.alloc_sbuf_tensor("sq_g", [P, D], F32).ap()
    s_sq = nc.alloc_sbuf_tensor("s_sq", [1, D], F32).ap()
    partial = nc.alloc_sbuf_tensor("partial", [1, 1], F32).ap()

    import contextlib

    _stack = contextlib.ExitStack()
    with (
        _stack,
        nc.psum_tensor([1, D], F32) as s_acc,
        nc.semaphore("amr_sem") as amr_sem,
        nc.semaphore("warm_sem") as warm_sem,
        nc.semaphore("gp_sem") as gp_sem,
        nc.semaphore("ssq_sem") as ssq_sem,
        nc.semaphore("norm_sem") as norm_sem,
        nc.semaphore("inv_sem") as inv_sem,
        nc.semaphore("mm_sem") as mm_sem,
        nc.semaphore("fin_sem") as fin_sem,
        nc.semaphore("out_sem") as out_sem,
        nc.Block() as block,
    ):
        dma_sems = [
            _stack.enter_context(nc.semaphore(f"dma{i}"))
            for i in range(len(DMA_PLAN))
        ]

        def dma_src(di):
            kind, ap, r0, m = dmas[di]
            return x[r0 : r0 + m * P, :].rearrange("(p n) d -> p n d", p=P)

        @block.sync
        def _(sync):
            for di, (kind, ap, r0, m) in enumerate(dmas):
                if kind == "hs":
                    sync.dma_start(out=ap, in_=dma_src(di)).then_inc(
                        dma_sems[di], 16
                    )
            # store from the warm sync HWDGE ring.  No out_sem wait: the
            # walrus epilogue's Sync DRAIN covers the in-flight store, so
            # the semaphore sweep overlaps the DMA flight (~1.3 us).
            sync.wait_ge(fin_sem, 1)
            sync.dma_start(
                out=out, in_=partial, single_packet=True
            ).then_inc(out_sem, 16)

        @block.gpsimd
        def _(gpsimd):
            # fire-and-forget 1-row warmup DMA: the SWDGE's first transfer
            # pays ~2.4 us of ucode-init + descriptor-gen latency before its
            # first packet; later transfers pipeline their gen behind the
            # streaming.  A tiny dummy first absorbs that latency so the
            # first REAL chunk's packets start earlier.
            gpsimd.dma_start(
                out=warm, in_=x[0:1, :].rearrange("(p n) d -> p n d", p=1)
            ).then_inc(warm_sem, 16)
            for di, (kind, ap, r0, m) in enumerate(dmas):
                if kind == "sw":
                    gpsimd.dma_start(out=ap, in_=dma_src(di)).then_inc(
                        dma_sems[di], 16
                    )
            # square+rowsum for the GP_TILES (gpsimd is otherwise idle from
            # here on).  Completion is signalled by a trailing nop, not by
            # then_inc on the accum op itself (accum writes land late).
            last_dma_waited = [-1]
            for t in sorted(GP_TILES):
                di, i, ap, kind2 = tiles[t]
                if di > last_dma_waited[0]:
                    gpsimd.wait_ge(dma_sems[di], 16)
                    last_dma_waited[0] = di
                apf = ap.bitcast(F32) if kind2 != "sw" else ap
                gpsimd.scalar_tensor_tensor(
                    out=sq_g[:, :],
                    in0=apf[:, i, :],
                    scalar=1.0,
                    in1=apf[:, i, :],
                    op0=mybir.AluOpType.mult,
                    op1=mybir.AluOpType.mult,
                    accum_out=ss[:, t : t + 1],
                )
                gpsimd.engine_nop().then_inc(gp_sem, 1)

        @block.scalar
        def _(scalar):
            # Dummy activations: pull the ACT table loads (Square/Sqrt sets)
            # into the DMA flight time instead of the first real use.
            scalar.activation(
                out=sq_a[:1, :1],
                in_=s_sq[:1, :1],
                func=mybir.ActivationFunctionType.Square,
            )
            scalar.activation(
                out=sq_a[:1, :1],
                in_=s_sq[:1, :1],
                func=mybir.ActivationFunctionType.Sqrt,
            )

            last_dma_waited = [-1]

            def tile_wait(t):
                di = tiles[t][0]
                if di > last_dma_waited[0]:
                    scalar.wait_ge(dma_sems[di], 16)
                    last_dma_waited[0] = di

            def squares(gi):
                gt0, gsize, kind = groups[gi]
                for t in range(gt0, gt0 + gsize):
                    if t == NT - 1:
                        # final tile: ACT squares the second free-dim half in
                        # parallel with DVE's first-half amr (shorter tail)
                        tile_wait(t)
                        di, i, ap, kind2 = tiles[t]
                        apf = ap.bitcast(F32) if kind2 != "sw" else ap
                        scalar.activation(
                            out=sq_a[:, : D // 2],
                            in_=apf[:, i, D // 2 :],
                            func=mybir.ActivationFunctionType.Square,
                            accum_out=ss_b,
                        ).then_inc(ssq_sem, 1)
                        continue
                    if _engine(t, NT) != "act":
                        continue
                    tile_wait(t)
                    di, i, ap, kind2 = tiles[t]
                    apf = ap.bitcast(F32) if kind2 != "sw" else ap
                    scalar.activation(
                        out=sq_a[:, :],
                        in_=apf[:, i, :],
                        func=mybir.ActivationFunctionType.Square,
                        accum_out=ss[:, t : t + 1],
                    ).then_inc(ssq_sem, 1)

            def sqrt(gi):
                gt0, gsize, kind = groups[gi]
                scalar.wait_ge(amr_sem, gi + 1)
                if any(t in GP_TILES for t in range(gt0, gt0 + gsize)):
                    n_gp = sum(1 for t in GP_TILES if t < gt0 + gsize)
                    scalar.wait_ge(gp_sem, n_gp)
                scalar.activation(
                    out=nrm[:, gt0 : gt0 + gsize],
                    in_=ss[:, gt0 : gt0 + gsize],
                    func=mybir.ActivationFunctionType.Sqrt,
                ).then_inc(norm_sem, 1)

            for gi in range(len(groups)):
                squares(gi)
                sqrt(gi)

            # epilogue: partial = sum_d s[d]^2 (single PSUM read on ACT; a
            # [1,D] f32 DMA-out costs ~0.65us more wire flight than this)
            scalar.wait_ge(mm_sem, 1)
            scalar.activation(
                out=s_sq,
                in_=s_acc.ap(),
                func=mybir.ActivationFunctionType.Square,
                accum_out=partial,
            ).then_inc(fin_sem, 1)

        @block.vector
        def _(vector):
            n_act = 0
            last_dma_waited = [-1]

            def tile_wait(t):
                di = tiles[t][0]
                if di > last_dma_waited[0]:
                    vector.wait_ge(dma_sems[di], 16)
                    last_dma_waited[0] = di

            def amrs(gi, on_first_dve=None):
                nonlocal n_act
                gt0, gsize, kind = groups[gi]
                need_ssq_wait = False
                for t in range(gt0, gt0 + gsize):
                    if t == NT - 1:
                        # final tile: DVE amrs the first half; ACT's second
                        # half lands in ss_b and is added here
                        tile_wait(t)
                        di, i, ap, kind2 = tiles[t]
                        apf = ap.bitcast(F32) if kind2 != "sw" else ap
                        vector.affine_mul_reduce(
                            out=sq_v[:, : D // 2],
                            accum_out=ss[:, t : t + 1],
                            in0=apf[:, i, : D // 2],
                            in1=apf[:, i, : D // 2],
                            scale=1.0,
                            bias=0.0,
                        )
                        n_act += 1  # ACT's half-square of this tile
                        vector.wait_ge(ssq_sem, n_act)
                        vector.tensor_add(
                            ss[:, t : t + 1], ss[:, t : t + 1], ss_b
                        )
                        continue
                    eng = _engine(t, NT)
                    if eng == "act":
                        n_act += 1
                        need_ssq_wait = True
                        continue
                    if eng == "gp":
                        continue
                    tile_wait(t)
                    di, i, ap, kind2 = tiles[t]
                    apf = ap.bitcast(F32) if kind2 != "sw" else ap
                    vector.affine_mul_reduce(
                        out=sq_v[:, :],
                        accum_out=ss[:, t : t + 1],
                        in0=apf[:, i, :],
                        in1=apf[:, i, :],
                        scale=1.0,
                        bias=0.0,
                    )
                    if on_first_dve is not None:
                        on_first_dve()
                        on_first_dve = None
                tile_wait(gt0 + gsize - 1)
                vector.engine_nop().then_inc(amr_sem, 1)
                if on_first_dve is not None:
                    on_first_dve()  # group had no plain DVE tile

            def recip(gi):
                gt0, gsize, kind = groups[gi]
                inv = inv_b if kind == "sw" else inv_r
                vector.wait_ge(norm_sem, gi + 1)
                with nc.allow_low_precision(reason="matmul weight dtype"):
                    vector.reciprocal(
                        out=inv[:, gt0 : gt0 + gsize],
                        in_=nrm[:, gt0 : gt0 + gsize],
                    ).then_inc(inv_sem, 1)

            # recip(g) is emitted after amrs(g+1) mid-stream: its wait on
            # ACT's sqrt(g) then never head-of-line blocks the amr stream,
            # so neither engine's chunk work waits on the other
            # (ping-pong-free).  For the last LAG0_TAIL groups the lag
            # collapses to zero: there the following amrs are data-gated on
            # the trailing 1-2 tile chunks anyway, and zero lag minimizes
            # the inv latency of the final tiles.
            LAG0_TAIL = 1  # lag-0 beyond the last group hurt: DVE enters
            #                the tail with ~1.3 us of backlog, so recip
            #                waits there still block its amr stream
            ng = len(groups)
            next_recip = 0

            def flush_recips(target):
                nonlocal next_recip
                while next_recip <= target:
                    recip(next_recip)
                    next_recip += 1

            for gi in range(ng):
                # the pending recip(gi-1) is emitted after the FIRST amr of
                # group gi (halves the recip-lag latency vs after all amrs)
                # lag-0 everywhere re-measured WORSE even on 4-tile chunks
                # (mmlag 6.0-6.8 vs 5.2): the recip's wait on sqrt(gi)
                # compounds across chunks.  Keep lag-1 with mid-group
                # emission; collapse only at the very last group.
                target = gi if gi >= ng - LAG0_TAIL else gi - 1
                amrs(gi, on_first_dve=lambda g=gi: flush_recips(g - 1))
                flush_recips(target)
            flush_recips(ng - 1)

        @block.tensor
        def _(tensor):
            mm = 0
            for gi, (gt0, gsize, kind) in enumerate(groups):
                inv = inv_b if kind == "sw" else inv_r
                tensor.wait_ge(inv_sem, gi + 1)
                for t in range(gt0, gt0 + gsize):
                    di, i, ap, kind2 = tiles[t]
                    instr = tensor.matmul(
                        s_acc.ap(),
                        inv[:, t : t + 1],
                        ap[:, i, :],
                        start=(mm == 0),
                        stop=(mm == NT - 1),
                    )
                    mm += 1
            instr.then_inc(mm_sem, 1)


_NC_CACHE = {}


def _get_nc():
    if "nc" not in _NC_CACHE:
        _NC_CACHE["nc"] = _build_nc()
    return _NC_CACHE["nc"]


def kernel(x: np.ndarray, trace: bool = False):
    assert x.shape == (B, N, D), x.shape
    nc = _get_nc()
    in_maps = [{"x": np.ascontiguousarray(x[b], dtype=np.float32)} for b in range(B)]
    res = None
    for attempt in range(3):
        try:
            res = run_bass_kernel_spmd(
                nc, in_maps, core_ids=list(range(B)), trace=trace
            )
            break
        except Exception:
            # A previously crashed process can leave the accelerator in an
            # "unrecoverable" state for ~30s; it heals on its own.
            if attempt == 2:
                raise
            import time

            time.sleep(25)
    partials = [float(r["out"][0, 0]) for r in res.results]
    val = np.float32(np.sum(np.asarray(partials, dtype=np.float64)) / (N * N) / B)
    if trace:
        return val, res
    return val
